# revision 21
# baseline (speedup 1.0000x reference)
"""DynamicConv1dTBC Trainium2 Bass kernel (v3 — barrier-amortized pipeline).

Problem: x [T=2048, B=4, C=1024] f32, Wlin [240, 1024] f32.
  w = softmax(einsum('tbc,kc->tbk', x, Wlin).reshape(T,B,H=16,K=15), axis=-1)
  out[t,b,h,r] = sum_k w[t,b,h,k] * xpad[t+k, b, h*64+r]   (causal, PAD_L=14)

Sharding: T split across 8 cores (256 out-timesteps each + 14-row left halo).

Measured HW findings this design is built around:
  - For_i executes an InstAllEngineBarrier in its per-iteration semaphore
    reset, so loop iterations CANNOT overlap: per-rep time equals the full
    serial span of one body. Fix: unroll 8 reps per iteration with a
    2-parity buffer rotation (x tiles + wpad planes) so reps pipeline
    inside the body and the barrier is paid once per 8 reps. Unroll 16
    regresses badly (instruction-fetch cliff); 4 and 12 are worse than 8.
  - f32->bf16 casting DMA is gpsimd/SWDGE-only and runs at ~190 GB/s
    HBM-side vs ~290-330 for plain f32 loads, but every on-chip cast
    alternative (gpsimd DSP copy, DVE copy, scalar-queue f32 loads) loses
    more on the compute engines than the ~8us the DMA saves; x is
    cast-loaded directly to bf16 on the SWDGE queue.
  - Stores: 3 wide [L, 16KB-row] stores per rep (all 4 b staged into one
    [128, B*C] f32 tile per conv chunk) beat 12 narrow per-(b,ci) stores
    by ~12us/rep. Per-core write throughput is ~190-390 GB/s depending on
    8-core contention; out traffic (4.2MB f32) is the largest single item.

Per-rep structure (one body; all constants hoisted out of the loop):
  A. weight-gen in 2 chunks of 128 out-rows: PE-transpose the three
     B-aligned x tiles in full (matmul base-partition rule forbids
     straddled reads), realign chunks in the ACT PSUM-evac copies, 8
     accumulating bf16 matmuls -> logits; Exp on ACT writes the packed
     [128, 256] wn row (per-head pitch 16, pad col zero); DVE reduce_sum +
     reciprocal, gpsimd normalize; packed row -> per-b DRAM plane
     [257, 256] (sync queue).
  B. per (b, chunk of {114,114,28} out rows): shear read (row pitch 255)
     of the plane yields the per-head band + junk; 8 PE transposes per
     half -> PSUM; DVE masked evac (0/1 diagonal-band mask) zeroes the
     junk; one bf16 conv matmul per head (s-window = L+14 <= 128); ACT
     evacs conv PSUM into the shared wide stage at the b-column offset;
     one wide store per chunk once all 4 b finish (sync queue).
"""
import sys, os
for _p in ("/opt/trn_rl_repo",):
    if _p not in sys.path and os.path.isdir(_p):
        sys.path.insert(0, _p)

import itertools
import numpy as np
from contextlib import ExitStack

import concourse.bass as bass
import concourse.tile as tile
from concourse import mybir, bacc, masks
from concourse._compat import with_exitstack
from concourse.bass_utils import run_bass_kernel_spmd

# ---- problem constants -------------------------------------------------------
T_GLOBAL, B, C = 2048, 4, 1024
H, K, R = 16, 15, 64
J = H * K                      # 240
PAD_L = K - 1                  # 14
N_CORES = 8
T_LOC = T_GLOBAL // N_CORES    # 256 output timesteps per core
T_EXT = T_LOC + PAD_L          # 270 input rows per core
HP = 16                        # per-head pitch inside a packed wn row
PITCH = H * HP                 # 256: wpad row pitch (elems, bf16) = 512B
RD = 384                       # anm read row width (covers 240 + 128 + pad)
# phase-B conv chunks: (t0, L); s-window = L+14 <= 128
CH = [(0, 114), (114, 114), (228, 28)]
XROW = [0, 114, 228]           # x tile start rows (ext time)
XLEN = [128, 128, 42]          # x tile heights = conv s-windows
# phase-A chunks: 128 out rows each = ext rows [14,142) and [142,270).
# Matmul operands must sit at base partition 0, so each x tile is PE-
# transposed in full; the chunk realignment happens in the PSUM-evac
# copies: (src ptx index, src col0, src col1, dst col0) per chunk.
ACH = [
    [(0, 14, 128, 0), (1, 14, 28, 114)],     # ch0 = xt0 cols[14:128] + xt1 cols[14:28]
    [(1, 28, 128, 0), (2, 14, 42, 100)],     # ch1 = xt1 cols[28:128] + xt2 cols[14:42]
]
F32 = mybir.dt.float32
BF16 = mybir.dt.bfloat16


@with_exitstack
def dynconv_program(ctx: ExitStack, tc: tile.TileContext,
                    x_ap: bass.AP, wlin_ap: bass.AP, out_ap: bass.AP,
                    planes: dict, reps: int):
    nc = tc.nc
    const = ctx.enter_context(tc.tile_pool(name="const", bufs=1))
    wl = ctx.enter_context(tc.tile_pool(name="wl", bufs=1))
    xp = ctx.enter_context(tc.tile_pool(name="xp", bufs=1))
    wnp = ctx.enter_context(tc.tile_pool(name="wnp", bufs=1))
    xtw = ctx.enter_context(tc.tile_pool(name="xtw", bufs=6))
    sump = ctx.enter_context(tc.tile_pool(name="sump", bufs=4))
    anmp = ctx.enter_context(tc.tile_pool(name="anmp", bufs=3))
    bandp = ctx.enter_context(tc.tile_pool(name="bandp", bufs=3))
    stg = ctx.enter_context(tc.tile_pool(name="stg", bufs=4))
    ps_at = ctx.enter_context(tc.tile_pool(name="psat", bufs=2, space="PSUM"))
    ps_aw = ctx.enter_context(tc.tile_pool(name="psaw", bufs=2, space="PSUM"))
    ps_bt = ctx.enter_context(tc.tile_pool(name="psbt", bufs=2, space="PSUM"))
    ps_bc = ctx.enter_context(tc.tile_pool(name="psbc", bufs=2, space="PSUM"))

    xv = x_ap.rearrange("t b c -> t (b c)")

    # ---- one-time constants --------------------------------------------------
    ident = const.tile([128, 128], BF16)
    masks.make_identity(nc, ident[:])

    wlin_b = wl.tile([120, C], BF16)
    nc.gpsimd.dma_start(wlin_b[:], wlin_ap[0:120, :])
    wlin_b2 = wl.tile([120, C], BF16)
    nc.gpsimd.dma_start(wlin_b2[:], wlin_ap[120:240, :])

    # WlinT: per-cchunk [128 c, 240 j] bf16 via PE transpose
    wlinT = []
    for cc in range(8):
        wt = wl.tile([128, J], BF16, name=f"wlinT{cc}", tag=f"wlinT{cc}")
        pt = ps_at.tile([128, 8 * 128], BF16, tag="ptx")
        for i, wb in enumerate((wlin_b, wlin_b2)):
            nc.tensor.matmul(pt[:, i * 120:(i + 1) * 120],
                             wb[:, cc * 128:(cc + 1) * 128],
                             ident[0:120, 0:120],
                             is_transpose=True, skip_group_check=True)
        nc.vector.tensor_copy(wt[:], pt[:, 0:J])
        wlinT.append(wt)

    # diagonal band mask [s, t]: 1 iff 0 <= s - t <= 15 (col 16h+15 is a
    # real zero in wn, so including s-t == 15 is harmless)
    maskb = const.tile([128, 114], BF16)
    nc.gpsimd.memset(maskb[:], 1.0)
    nc.gpsimd.affine_select(maskb[:], maskb[:], pattern=[[-1, 114]],
                            compare_op=mybir.AluOpType.is_ge, fill=0.0,
                            base=0, channel_multiplier=1)
    nc.gpsimd.affine_select(maskb[:], maskb[:], pattern=[[1, 114]],
                            compare_op=mybir.AluOpType.is_ge, fill=0.0,
                            base=K, channel_multiplier=-1)

    # wn buffers [128, 256] bf16; pad col 16h+15 zeroed once, then kept zero
    # by the normalize pass (0 * inv = 0)
    wn_bufs = []
    for i in range(6):
        wz = wnp.tile([128, PITCH], BF16, name=f"wn{i}", tag=f"wn{i}")
        nc.gpsimd.memset(wz[:], 0.0)
        wn_bufs.append(wz)

    # 2 parity sets of x tiles
    xt = [[xp.tile([XLEN[i], B * C], BF16, name=f"xt{p}_{i}", tag=f"xt{p}_{i}")
           for i in range(3)] for p in range(2)]

    wn_cyc = itertools.count()

    # ---- phase A: weight-gen per (b, ch of 128 out rows) ---------------------
    def transpose_xt(p, b, xi):
        n = XLEN[xi]
        ptx = ps_at.tile([128, 8 * 128], BF16, tag="ptx")
        for cc in range(8):
            nc.tensor.matmul(
                ptx[:, cc * n:(cc + 1) * n],
                xt[p][xi][0:n, b * C + cc * 128: b * C + (cc + 1) * 128],
                ident[0:n, 0:n],
                is_transpose=True, skip_group_check=True)
        return ptx

    def evac_piece(xTw, ch, piece_i, ptx):
        # copy one ACH piece of a transposed x tile into the chunk's xTw
        (pi, c0, c1, d0) = ACH[ch][piece_i]
        n = c1 - c0
        src_n = XLEN[pi]
        nc.scalar.activation(
            xTw[:].rearrange("c (cc t) -> c cc t", t=128)[:, :, d0:d0 + n],
            ptx[:, 0:8 * src_n]
            .rearrange("c (cc t) -> c cc t", t=src_n)[:, :, c0:c1],
            mybir.ActivationFunctionType.Copy)

    def phase_a(p, b, ch, xTw):
        pw = ps_aw.tile([128, J], F32)
        for cc in range(8):
            nc.tensor.matmul(pw[:, :], xTw[:, cc * 128:(cc + 1) * 128],
                             wlinT[cc][:], start=(cc == 0), stop=(cc == 7))
        wn = wn_bufs[next(wn_cyc) % 6]
        nc.scalar.activation(
            wn[:].rearrange("t (h k) -> t h k", k=HP)[:, :, 0:K],
            pw[:].rearrange("t (h k) -> t h k", k=K),
            mybir.ActivationFunctionType.Exp)
        sums = sump.tile([128, H], F32, tag="sums")
        nc.vector.reduce_sum(sums[:],
                             wn[:].rearrange("t (h k) -> t h k", k=HP),
                             axis=mybir.AxisListType.X)
        inv = sump.tile([128, H], F32, tag="inv")
        nc.vector.reciprocal(inv[:], sums[:])
        nc.gpsimd.tensor_tensor(
            wn[:].rearrange("t (h k) -> t h k", k=HP),
            wn[:].rearrange("t (h k) -> t h k", k=HP),
            inv[:].unsqueeze(2).broadcast_to((128, H, HP)),
            op=mybir.AluOpType.mult)
        wpd = planes[(p, b)][:]
        dst = bass.AP(wpd.tensor, wpd.offset + ch * 128 * PITCH,
                      [[PITCH, 128], [1, PITCH]])
        nc.sync.dma_start(dst, wn[:])

    # ---- phase B: shear read + banded transposes + conv per (b, ci) ----------
    def phase_b(p, b, ci, stage):
        t0, L = CH[ci]
        SW_ = L + PAD_L
        wpd = planes[(p, b)][:]
        rd = H * K + SW_
        anm = anmp.tile([128, RD], BF16, tag="anm")
        src = bass.AP(wpd.tensor, wpd.offset + t0 * PITCH,
                      [[PITCH - 1, L], [1, rd]])
        nc.sync.dma_start(anm[0:L, 0:rd], src)
        for hq in range(2):
            ptb = ps_bt.tile([128, 8 * 114], BF16, tag="ptb")
            for j in range(8):
                h = hq * 8 + j
                nc.tensor.matmul(
                    ptb[0:SW_, j * L:(j + 1) * L],
                    anm[0:L, HP * h: HP * h + SW_],
                    ident[0:L, 0:L],
                    is_transpose=True, skip_group_check=True)
            band = bandp.tile([128, 8 * 114], BF16, tag="band")
            nc.vector.tensor_tensor(
                band[0:SW_, 0:8 * L].rearrange("s (j t) -> s j t", t=L),
                ptb[0:SW_, 0:8 * L].rearrange("s (j t) -> s j t", t=L),
                maskb[0:SW_, 0:L].unsqueeze(1).broadcast_to((SW_, 8, L)),
                op=mybir.AluOpType.mult)
            pc = ps_bc.tile([128, 8 * R], F32, tag="pc")
            for j in range(8):
                h = hq * 8 + j
                nc.tensor.matmul(
                    pc[0:L, j * R:(j + 1) * R],
                    band[0:SW_, j * L:(j + 1) * L],
                    xt[p][ci][0:SW_, b * C + h * R: b * C + (h + 1) * R],
                    start=True, stop=True, skip_group_check=True)
            nc.scalar.activation(
                stage[0:L, b * C + hq * 8 * R: b * C + (hq + 1) * 8 * R],
                pc[0:L, 0:8 * R],
                mybir.ActivationFunctionType.Copy)

    # ---- one rep -------------------------------------------------------------
    def body(p):
        for i in range(3):
            t0, n = XROW[i], XLEN[i]
            nc.gpsimd.dma_start(xt[p][i][:], xv[t0:t0 + n, :])
        # stage 1: xt0 transposes for all b — PE busy as soon as xt0 lands;
        # each ptx is evac'd immediately so the 2 PSUM bufs recycle.
        xTw0 = {}
        for b in range(B):
            ptx = transpose_xt(p, b, 0)
            xTw0[b] = xtw.tile([128, 8 * 128], BF16, name="xTw0", tag="xTw")
            evac_piece(xTw0[b], 0, 0, ptx)
        # stage 2 (needs xt1): finish ch0 per b, start ch1's first piece
        xTw1 = {}
        for b in range(B):
            ptx = transpose_xt(p, b, 1)
            evac_piece(xTw0[b], 0, 1, ptx)
            xTw1[b] = xtw.tile([128, 8 * 128], BF16, name="xTw1", tag="xTw")
            evac_piece(xTw1[b], 1, 0, ptx)
            phase_a(p, b, 0, xTw0[b])
        # stage 3 (needs xt2): finish ch1 per b, then phase B as planes land
        for b in range(B):
            ptx = transpose_xt(p, b, 2)
            evac_piece(xTw1[b], 1, 1, ptx)
            phase_a(p, b, 1, xTw1[b])
        ov = out_ap.rearrange("t b c -> t (b c)")
        for ci in range(3):
            t0, L = CH[ci]
            stage = stg.tile([128, B * C], F32, tag="stage")
            for b in range(B):
                phase_b(p, b, ci, stage)
            nc.sync.dma_start(ov[t0:t0 + L, :], stage[0:L, :])

    if reps == 1:
        body(0)
    else:
        unroll = int(os.environ.get("KM_UNROLL", "8"))
        assert reps % unroll == 0, f"reps must be divisible by {unroll}"
        with tc.For_i(0, reps // unroll, 1):
            for r in range(unroll):
                body(r % 2)


def build_program(debug=False, reps=1):
    nc = bacc.Bacc("TRN2", target_bir_lowering=False, debug=debug,
                   enable_asserts=False, num_devices=N_CORES)
    x_t = nc.dram_tensor("x", [T_EXT, B, C], F32, kind="ExternalInput")
    wlin_t = nc.dram_tensor("wlin", [J, C], F32, kind="ExternalInput")
    out_t = nc.dram_tensor("out", [T_LOC, B, C], F32, kind="ExternalOutput")
    with tile.TileContext(nc) as tc:
        with tc.tile_pool(name="wpddram", bufs=1, space="DRAM") as wpool:
            planes = {(p, b): wpool.tile([257, PITCH], BF16,
                                         name=f"wpd{p}_{b}", tag=f"wpd{p}_{b}")
                      for p in range(2) for b in range(B)}
            dynconv_program(tc, x_t.ap(), wlin_t.ap(), out_t.ap(),
                            planes, reps)
    nc.compile()
    return nc


_NC_CACHE = None


def kernel(x: np.ndarray, Wlin: np.ndarray) -> np.ndarray:
    global _NC_CACHE
    if _NC_CACHE is None:
        _NC_CACHE = build_program()
    nc = _NC_CACHE
    xp = np.pad(x, ((PAD_L, 0), (0, 0), (0, 0)))
    in_maps = []
    for i in range(N_CORES):
        in_maps.append({
            "x": np.ascontiguousarray(xp[i * T_LOC:i * T_LOC + T_EXT]),
            "wlin": np.ascontiguousarray(Wlin),
        })
    res = run_bass_kernel_spmd(nc, in_maps, core_ids=list(range(N_CORES)))
    outs = [res.results[i]["out"] for i in range(N_CORES)]
    return np.concatenate(outs, axis=0)
